# revision 30
# baseline (speedup 1.0000x reference)
"""NeuTraLAD loss kernel for Trainium2, 8-core data parallel.

Shapes (hardcoded): x [16384, 512], K=11 transforms of 3x[512,512] MLPs,
shared 3-layer encoder + LayerNorm, cosine-sim contrastive loss -> [16384].

Strategy: shard batch across 8 cores (2048 rows each). Inside each core,
feature-major dataflow: activations live as [128 part = feature block,
batch free dim], weights are lhsT blocks, so the whole 6-layer chain runs
with zero transposes. LayerNorm / cosine-norm reductions over features are
partition sums done with ones-vector matmuls on the PE; per-sample pair
dots (66 of them) are DVE elementwise muls + ones-matmul partition sums;
logsumexp denominators via one [66->11] selection matmul + Ln.
"""

import numpy as np
from contextlib import ExitStack

import concourse.bass as bass
import concourse.bacc as bacc
import concourse.mybir as mybir
import concourse.tile as tile
from concourse.bass_utils import run_bass_kernel_spmd

AF = mybir.ActivationFunctionType
ALU = mybir.AluOpType
F32 = mybir.dt.float32
F32R = mybir.dt.float32r
BF16 = mybir.dt.bfloat16

B, D, K = 16384, 512, 11
NCORES = 8
BC = B // NCORES          # 2048 rows per core
NB = 512                  # batch tile (matmul moving free dim)
NT = BC // NB             # 4 batch tiles per core
HB = D // 128             # 4 feature blocks of 128
NV = K + 1                # 11 zkn + zn
# pair r: (K, k) = pos_k for k<11 ; then (l, k) l<k = S[l,k]
PAIRS = [(K, k) for k in range(K)] + [
    (l, k) for l in range(K) for k in range(l + 1, K)
]
NPAIR = len(PAIRS)        # 66
LN_EPS = 1e-5
COS_EPS = 1e-8


def _sel_matrix() -> np.ndarray:
    """selc[r, kk] = 1 if pair r contributes to denominator kk."""
    sel = np.zeros((NPAIR, K), np.float32)
    for r, (a, b) in enumerate(PAIRS):
        if a == K:
            sel[r, b] = 1.0       # pos_k only in denominator k
        else:
            sel[r, a] = 1.0       # S[l,k] symmetric: denominators l and k
            sel[r, b] = 1.0
    return sel


def _build_program():
    nc = bacc.Bacc("TRN2", target_bir_lowering=False, debug=False)

    xT = nc.declare_dram_parameter("xT", [HB, 128, BC], F32, False)
    tW1 = nc.declare_dram_parameter("tW1", [K, HB, 128, D], F32, False)
    tW2 = nc.declare_dram_parameter("tW2", [K, HB, 128, D], F32, False)
    tW3 = nc.declare_dram_parameter("tW3", [K, HB, 128, D], F32, False)
    tb1 = nc.declare_dram_parameter("tb1", [K, HB, 128, 1], F32, False)
    tb2 = nc.declare_dram_parameter("tb2", [K, HB, 128, 1], F32, False)
    tb3 = nc.declare_dram_parameter("tb3", [K, HB, 128, 1], F32, False)
    eW1 = nc.declare_dram_parameter("eW1", [HB, 128, D], F32, False)
    eW2 = nc.declare_dram_parameter("eW2", [HB, 128, D], F32, False)
    eW3 = nc.declare_dram_parameter("eW3", [HB, 128, D], F32, False)
    eb1 = nc.declare_dram_parameter("eb1", [HB, 128, 1], F32, False)
    eb2 = nc.declare_dram_parameter("eb2", [HB, 128, 1], F32, False)
    eb3 = nc.declare_dram_parameter("eb3", [HB, 128, 1], F32, False)
    ln_g = nc.declare_dram_parameter("ln_g", [HB, 128, 1], F32, False)
    ln_b = nc.declare_dram_parameter("ln_b", [HB, 128, 1], F32, False)
    selc = nc.declare_dram_parameter("selc", [NPAIR, K], F32, False)
    ones_d = nc.declare_dram_parameter("ones_d", [128, 1], F32, False)
    y = nc.declare_dram_parameter("y", [NT, 1, NB], F32, True)

    with tile.TileContext(nc) as tc, ExitStack() as ctx:
        const = ctx.enter_context(tc.tile_pool(name="const", bufs=1))
        wenc = ctx.enter_context(tc.tile_pool(name="wenc", bufs=1))
        wstr = ctx.enter_context(tc.tile_pool(name="wstr", bufs=1))
        xpool = ctx.enter_context(tc.tile_pool(name="xpool", bufs=2))
        hpool = ctx.enter_context(tc.tile_pool(name="hpool", bufs=1))
        zpool = ctx.enter_context(tc.tile_pool(name="zpool", bufs=NV))
        spool = ctx.enter_context(tc.tile_pool(name="spool", bufs=2))
        ppool = ctx.enter_context(tc.tile_pool(name="ppool", bufs=3))
        psA = ctx.enter_context(tc.tile_pool(name="psA", bufs=2, space="PSUM"))
        psB = ctx.enter_context(tc.tile_pool(name="psB", bufs=3, space="PSUM"))
        psC = ctx.enter_context(tc.tile_pool(name="psC", bufs=2, space="PSUM"))
        psD = ctx.enter_context(tc.tile_pool(name="psD", bufs=1, space="PSUM"))

        # ---- constants ----
        ones128 = const.tile([128, 1], F32R)
        nc.sync.dma_start(ones128[:], ones_d[:].bitcast(F32R))
        ones128b = const.tile([128, 1], BF16)
        nc.vector.memset(ones128b[:], 1.0)
        ones_row = const.tile([1, 128], F32)
        nc.vector.memset(ones_row[:], 1.0)
        ones11 = const.tile([K, 1], F32)
        nc.vector.memset(ones11[:], 1.0)
        neg11 = const.tile([K, 1], F32)
        nc.vector.memset(neg11[:], -1.0)
        sel_sb = const.tile([NPAIR, K], F32)
        nc.sync.dma_start(sel_sb[:], selc[:])
        eps1 = const.tile([1, 1], F32)
        nc.vector.memset(eps1[:], LN_EPS)

        # ---- resident weights / biases ----
        ew = []
        for name, wd in (("ew1", eW1), ("ew2", eW2), ("ew3", eW3)):
            w = wenc.tile([128, HB * D], F32R, name=name)
            for ib in range(HB):
                nc.sync.dma_start(w[:, ib * D:(ib + 1) * D],
                                  wd[ib].bitcast(F32R))
            ew.append(w)

        def load_bias_cols(name, pool, dram, ncols, idx):
            t = pool.tile([128, ncols], F32, name=name)
            for c in range(ncols):
                nc.sync.dma_start(t[:, c:c + 1], dram[idx + (c,)])
            return t

        eb = [load_bias_cols(f"eb{i}", const, d, HB, ())
              for i, d in enumerate((eb1, eb2, eb3))]
        g_sb = load_bias_cols("g_sb", const, ln_g, HB, ())
        b_sb = load_bias_cols("b_sb", const, ln_b, HB, ())
        # all transform biases resident: [128, K*HB], col k*HB+jb
        tb = []
        for i, d in enumerate((tb1, tb2, tb3)):
            t = const.tile([128, K * HB], F32, name=f"tb{i}")
            for k in range(K):
                for jb in range(HB):
                    nc.sync.dma_start(t[:, k * HB + jb:k * HB + jb + 1],
                                      d[k, jb])
            tb.append(t)

        # ---- helpers ----
        def mlp_layer(in_sb, w_sb, bias_ap_fn, func, out_name):
            out_sb = hpool.tile([128, HB * NB], F32R, name=out_name)
            for jb in range(HB):
                ps = psA.tile([128, NB], F32, name="mm")
                for ib in range(HB):
                    nc.tensor.matmul(
                        ps[:],
                        w_sb[:, ib * D + jb * 128: ib * D + (jb + 1) * 128],
                        in_sb[:, ib * NB:(ib + 1) * NB],
                        start=(ib == 0), stop=(ib == HB - 1),
                    )
                nc.scalar.activation(out_sb[:, jb * NB:(jb + 1) * NB], ps[:],
                                     func, bias=bias_ap_fn(jb))
            return out_sb

        def part_sums(src_sb, name):
            """[1, NB] psum = column sums over all 512 feature partitions."""
            ps = psB.tile([1, NB], F32, name=name)
            for hb in range(HB):
                nc.tensor.matmul(ps[:], ones128[:],
                                 src_sb[:, hb * NB:(hb + 1) * NB],
                                 start=(hb == 0), stop=(hb == HB - 1))
            return ps

        def bcast(row_ap, name):
            """[128, NB] psum = row broadcast across partitions (f32)."""
            ps = psC.tile([128, NB], F32, name=name)
            nc.tensor.matmul(ps[:], ones_row[:], row_ap,
                             start=True, stop=True)
            return ps

        def sq_part_sums(src_sb, name):
            """[1, NB] psum = column sums of src**2 over 512 partitions."""
            ps = psB.tile([1, NB], F32, name=name)
            for hb in range(HB):
                zt = hpool.tile([128, NB], F32R, name="zsq", bufs=2)
                nc.scalar.activation(zt[:], src_sb[:, hb * NB:(hb + 1) * NB],
                                     AF.Square)
                nc.tensor.matmul(ps[:], ones128[:], zt[:],
                                 start=(hb == 0), stop=(hb == HB - 1))
            return ps

        def encoder(in_sb, zdst):
            h1 = mlp_layer(in_sb, ew[0], lambda jb: eb[0][:, jb:jb + 1],
                           AF.Gelu, "eh1")
            h2 = mlp_layer(h1, ew[1], lambda jb: eb[1][:, jb:jb + 1],
                           AF.Gelu, "eh2")
            z3 = mlp_layer(h2, ew[2], lambda jb: eb[2][:, jb:jb + 1],
                           AF.Identity, "z3")
            # LN stats over features (partition sums via PE)
            ps_s = part_sums(z3, "st")
            ps_q = sq_part_sums(z3, "st")
            # all [1,NB] stat rows at partition 0 (DVE needs equal bases)
            mean = spool.tile([1, NB], F32, name="mean")[:]
            nc.scalar.activation(mean, ps_s[:], AF.Copy, scale=1.0 / D)
            var = spool.tile([1, NB], F32, name="aux")[:]
            nc.vector.tensor_mul(var, mean, mean)      # mean^2
            # var = ps_q/D - mean^2   (one PSUM read, one SBUF read)
            nc.vector.scalar_tensor_tensor(var, ps_q[:], 1.0 / D, var,
                                           ALU.mult, ALU.subtract)
            std = spool.tile([1, NB], F32, name="aux")[:]
            nc.scalar.activation(std, var, AF.Sqrt, bias=eps1[:])
            rstd = spool.tile([1, NB], F32, name="rcp")[:]
            nc.vector.reciprocal(rstd, std)
            c_b = bcast(mean, "bc")
            r_b = bcast(rstd, "bc")
            zl = hpool.tile([128, HB * NB], F32, name="zl")
            for hb in range(HB):
                sl = slice(hb * NB, (hb + 1) * NB)
                nc.vector.tensor_sub(zl[:, sl], z3[:, sl].bitcast(F32), c_b[:])
                nc.vector.scalar_tensor_tensor(
                    zl[:, sl], zl[:, sl], g_sb[:, hb:hb + 1], r_b[:],
                    ALU.mult, ALU.mult)
                nc.vector.tensor_scalar_add(zl[:, sl], zl[:, sl],
                                            b_sb[:, hb:hb + 1])
            # cosine normalize
            ps_n = sq_part_sums(zl, "st")
            nrm = spool.tile([1, NB], F32, name="aux")[:]
            nc.scalar.activation(nrm, ps_n[:], AF.Sqrt)
            nc.vector.tensor_scalar_max(nrm, nrm, COS_EPS)
            rn = spool.tile([1, NB], F32, name="rcp")[:]
            nc.vector.reciprocal(rn, nrm)
            rn_b = bcast(rn, "bc")
            for hb in range(HB):
                sl = slice(hb * NB, (hb + 1) * NB)
                nc.vector.tensor_mul(zdst[:, sl], zl[:, sl], rn_b[:])

        # ---- main loop over batch tiles ----
        for t in range(NT):
            x_sb = xpool.tile([128, HB * NB], F32R, name="x_sb")
            for hb in range(HB):
                nc.sync.dma_start(x_sb[:, hb * NB:(hb + 1) * NB],
                                  xT[hb, :, t * NB:(t + 1) * NB].bitcast(F32R))
            zvecs = [None] * NV
            zvecs[K] = zpool.tile([128, HB * NB], BF16, name="zkn")
            encoder(x_sb, zvecs[K])
            for k in range(K):
                tw = []
                for i, wd in enumerate((tW1, tW2, tW3)):
                    w = wstr.tile([128, HB * D], F32R, name=f"tw{i}")
                    for ib in range(HB):
                        nc.sync.dma_start(w[:, ib * D:(ib + 1) * D],
                                          wd[k, ib].bitcast(F32R))
                    tw.append(w)
                h1 = mlp_layer(x_sb, tw[0],
                               lambda jb: tb[0][:, k * HB + jb:k * HB + jb + 1],
                               AF.Gelu, "th1")
                h2 = mlp_layer(h1, tw[1],
                               lambda jb: tb[1][:, k * HB + jb:k * HB + jb + 1],
                               AF.Gelu, "th2")
                tx = mlp_layer(h2, tw[2],
                               lambda jb: tb[2][:, k * HB + jb:k * HB + jb + 1],
                               AF.Identity, "tx")
                zvecs[k] = zpool.tile([128, HB * NB], BF16, name="zkn")
                encoder(tx, zvecs[k])

            # ---- pair dots -> exp; DMA-scatter rows (engines can't
            # address partitions off quadrant bases, DMAs can) ----
            expd = spool.tile([NPAIR, NB], F32, name="gram", bufs=1)
            posr = spool.tile([K, NB], F32, name="posr", bufs=1)
            for r, (a, b) in enumerate(PAIRS):
                ps_d = psB.tile([1, NB], F32, name="st")
                for hb in range(HB):
                    sl = slice(hb * NB, (hb + 1) * NB)
                    pr = ppool.tile([128, NB], BF16, name="prod")
                    nc.vector.tensor_mul(pr[:], zvecs[a][:, sl],
                                         zvecs[b][:, sl])
                    nc.tensor.matmul(ps_d[:], ones128b[:], pr[:],
                                     start=(hb == 0), stop=(hb == HB - 1))
                ex_t = spool.tile([1, NB], F32, name="ex_t", bufs=3)
                nc.scalar.activation(ex_t[:], ps_d[:], AF.Exp)
                nc.sync.dma_start(expd[r:r + 1, :], ex_t[:])
                if r < K:
                    po_t = spool.tile([1, NB], F32, name="po_t", bufs=2)
                    nc.scalar.activation(po_t[:], ps_d[:], AF.Copy)
                    nc.sync.dma_start(posr[r:r + 1, :], po_t[:])

            # ---- logsumexp + loss ----
            ps_den = psD.tile([K, NB], F32, name="den")
            nc.tensor.matmul(ps_den[:], sel_sb[:], expd[:],
                             start=True, stop=True)
            ld = spool.tile([K, NB], F32, name="ld", bufs=1)
            nc.scalar.activation(ld[:], ps_den[:], AF.Ln)
            ps_loss = psB.tile([1, NB], F32, name="st")
            nc.tensor.matmul(ps_loss[:], ones11[:], ld[:],
                             start=True, stop=False)
            nc.tensor.matmul(ps_loss[:], neg11[:], posr[:],
                             start=False, stop=True)
            loss_sb = spool.tile([1, NB], F32, name="loss", bufs=1)
            nc.vector.tensor_copy(loss_sb[:], ps_loss[:])
            nc.sync.dma_start(y[t], loss_sb[:])

    nc.compile()
    return nc


_NC_CACHE = None


def _get_program():
    global _NC_CACHE
    if _NC_CACHE is None:
        _NC_CACHE = _build_program()
    return _NC_CACHE


def _make_in_maps(inputs):
    f = lambda a: np.ascontiguousarray(np.asarray(a, np.float32))
    shared = {
        "tW1": f(inputs["tW1"]).reshape(K, HB, 128, D),
        "tW2": f(inputs["tW2"]).reshape(K, HB, 128, D),
        "tW3": f(inputs["tW3"]).reshape(K, HB, 128, D),
        "tb1": f(inputs["tb1"]).reshape(K, HB, 128, 1),
        "tb2": f(inputs["tb2"]).reshape(K, HB, 128, 1),
        "tb3": f(inputs["tb3"]).reshape(K, HB, 128, 1),
        "eW1": f(inputs["eW1"]).reshape(HB, 128, D),
        "eW2": f(inputs["eW2"]).reshape(HB, 128, D),
        "eW3": f(inputs["eW3"]).reshape(HB, 128, D),
        "eb1": f(inputs["eb1"]).reshape(HB, 128, 1),
        "eb2": f(inputs["eb2"]).reshape(HB, 128, 1),
        "eb3": f(inputs["eb3"]).reshape(HB, 128, 1),
        "ln_g": f(inputs["ln_g"]).reshape(HB, 128, 1),
        "ln_b": f(inputs["ln_b"]).reshape(HB, 128, 1),
        "selc": _sel_matrix(),
        "ones_d": np.ones((128, 1), np.float32),
    }
    xT_full = np.ascontiguousarray(f(inputs["x"]).T)  # [512, 16384]
    in_maps = []
    for i in range(NCORES):
        m = dict(shared)
        m["xT"] = np.ascontiguousarray(
            xT_full[:, i * BC:(i + 1) * BC]).reshape(HB, 128, BC)
        in_maps.append(m)
    return in_maps


def run(inputs, trace=False):
    nc = _get_program()
    res = run_bass_kernel_spmd(nc, _make_in_maps(inputs),
                               list(range(NCORES)), trace=trace)
    out = np.concatenate([res.results[i]["y"].reshape(BC)
                          for i in range(NCORES)])
    return out.astype(np.float32), res


def kernel(**inputs):
    out, _ = run(inputs)
    return out
